# revision 43
# baseline (speedup 1.0000x reference)
"""Trainium2 Bass kernel for a dense transformer block (B=2, T=2048, C=1024, H=16).

Sharding across 8 NeuronCores:
  - LayerNorm1 computed token-sharded (512 tokens/core), AllGather of h.
  - Attention tensor-parallel over heads (2 heads/core): QKV projections,
    causal softmax, P@V all local per head.
  - AllToAll converts head-sharded y to token-sharded full-channel y.
  - Output projection Wp, LayerNorm2 and the whole MLP are token-sharded
    (full weights per core), so no further communication is needed.

Everything on-chip is kept channel-major ([C, tokens], C on partitions) so no
transposes are ever needed: weights are pre-transposed on the host, LayerNorm
statistics are computed with an all-ones matmul on the tensor engine (which
also broadcasts the per-token stats across all 128 partitions for free), and
the softmax denominator comes from 64 replicated ones-columns appended to V.
Matmul inputs are bf16 (fp32 accumulation in PSUM); the residual path stays
fp32 end to end.
"""

import sys

sys.path.insert(0, "/opt/trn_rl_repo")

import numpy as np
import ml_dtypes

import concourse.bass as bass
import concourse.bacc as bacc
import concourse.tile as tile
import concourse.mybir as mybir
from concourse import bass_utils

B, T, C, H = 2, 2048, 1024, 16
HD = C // H          # 64
FF = 4 * C           # 4096
EPS = 1e-5
NC = 8               # cores
P = 128
SH = (B * T) // NC   # 512 tokens per shard
KT = C // P          # 8 k-subtiles over C
FT = FF // P         # 32 ff tiles
TTILES = (B * T) // P  # 32 global 128-token tiles
CPB = T // SH        # 4 chunks per batch
f32 = mybir.dt.float32
bf16 = mybir.dt.bfloat16
BF = ml_dtypes.bfloat16

_CACHE = {}


def _build(stub_collectives=False):
    nc = bacc.Bacc("TRN2", target_bir_lowering=False, debug=False,
                   num_devices=1 if stub_collectives else NC)
    A = mybir.ActivationFunctionType
    OP = mybir.AluOpType

    def dram_in(name, shape, dt):
        return nc.dram_tensor(name, shape, dt, kind="ExternalInput").ap()

    xT = dram_in("xT", [P, KT, SH], f32)      # C-major token shard, k-blocked
    wqT = dram_in("wqT", [P, KT, P], bf16)    # [ki, ko, M=128 q-ch] blocked
    wkT = dram_in("wkT", [P, KT, P], bf16)
    wvT = dram_in("wvT", [P, KT, P], bf16)
    wpT = dram_in("wpT", [P, KT, C], bf16)    # [ki, ko, M=C] blocked (full)
    w1T = dram_in("w1T", [FT, P, KT, P], bf16)  # per ff-tile: [ki, ko, 128]
    w2T = dram_in("w2T", [KT, P, FT, P], bf16)  # per C-tile: [ki, ko, 128]
    bqk = dram_in("bqk", [P, 2], f32)         # [:,0]=bq slice, [:,1]=bk slice
    bv = dram_in("bv", [1, P], f32)           # bv slice (free-axis add)
    bp = dram_in("bp", [P, KT], f32)
    b1 = dram_in("b1", [P, FT], f32)
    b2 = dram_in("b2", [P, KT], f32)
    ln1w = dram_in("ln1w", [P, KT], f32)
    ln1b = dram_in("ln1b", [P, KT], f32)
    ln2w = dram_in("ln2w", [P, KT], f32)
    ln2b = dram_in("ln2b", [P, KT], f32)
    masks = dram_in("masks", [P, CPB, SH], bf16)  # [i, d, j] = (128d+i) <= j

    outT = nc.dram_tensor("outT", [C, SH], f32, kind="ExternalOutput").ap()

    rg = [list(range(NC))]

    def blocked(ap, ki=P):
        # [ (ko ki), s ] -> [ ki, ko, s ]  (channel c = 128*ko + ki)
        return ap.rearrange("(ko ki) s -> ki ko s", ki=ki)

    with tile.TileContext(nc) as tc:
        with (
            tc.tile_pool(name="dram", bufs=1, space="DRAM") as dram,
            tc.tile_pool(name="const", bufs=1) as const,
            tc.tile_pool(name="persist", bufs=1) as persist,
            tc.tile_pool(name="temps", bufs=3) as temps,
            tc.tile_pool(name="psum_y", bufs=2, space="PSUM") as psum_y,
        ):
            ag_in = dram.tile([C, SH], bf16)
            ag_out = dram.tile([NC * C, SH], bf16,
                               addr_space="Local" if stub_collectives else "Shared")
            a2a_in = dram.tile([NC * P, SH], bf16)
            a2a_out = dram.tile([NC * P, SH], bf16)

            ones_bf = const.tile([P, P], bf16)
            nc.vector.memset(ones_bf[:], 1.0)
            eps_t = const.tile([P, 1], f32)
            nc.vector.memset(eps_t[:], EPS)
            bqk_t = const.tile([P, 2], f32)
            nc.sync.dma_start(bqk_t[:], bqk[:])
            bv_rep = const.tile([P, P], f32)
            nc.sync.dma_start(
                bv_rep[:],
                bass.AP(tensor=bv.tensor, offset=bv.offset, ap=[[0, P], [1, P]]),
            )
            bp_t = const.tile([P, KT], f32)
            nc.sync.dma_start(bp_t[:], bp[:])
            b1_t = const.tile([P, FT], f32)
            nc.sync.dma_start(b1_t[:], b1[:])
            b2_t = const.tile([P, KT], f32)
            nc.sync.dma_start(b2_t[:], b2[:])
            lnp = {}
            for nm, ap in (("ln1w", ln1w), ("ln1b", ln1b), ("ln2w", ln2w), ("ln2b", ln2b)):
                t = const.tile([P, KT], f32, tag=nm)
                nc.sync.dma_start(t[:], ap[:])
                lnp[nm] = t
            mask_t = const.tile([P, CPB, SH], bf16)
            nc.scalar.dma_start(mask_t[:], masks[:])

            xT_sb = persist.tile([P, KT, SH], f32)
            for k in range(KT):
                nc.sync.dma_start(xT_sb[:, k, :], xT[:, k, :])

            def ln_stats_feed(s1, s2, x_ap, k):
                """Feed one [P, SH] fp32 tile into the LN stat accumulators.

                All-ones matmuls both sum over the C partition axis and
                broadcast the per-token result to every partition of the
                PSUM accumulators."""
                xbf = temps.tile([P, SH], bf16, tag="ln_xbf")
                nc.scalar.activation(xbf[:], x_ap, A.Copy)
                nc.tensor.matmul(s1[:], ones_bf[:], xbf[:], start=(k == 0), stop=(k == KT - 1))
                sq = temps.tile([P, SH], bf16, tag="ln_sq")
                nc.vector.tensor_mul(sq[:], xbf[:], xbf[:])
                nc.tensor.matmul(s2[:], ones_bf[:], sq[:], start=(k == 0), stop=(k == KT - 1))

            def ln_finalize(s1, s2, x_sb, w_t, b_t, out_writer):
                mean = temps.tile([P, SH], f32, tag="ln_mean")
                nc.vector.tensor_scalar_mul(mean[:], s1[:], 1.0 / C)
                var = temps.tile([P, SH], f32, tag="ln_var")
                nc.vector.tensor_scalar_mul(var[:], s2[:], 1.0 / C)
                msq = temps.tile([P, SH], f32, tag="ln_t")
                nc.vector.tensor_mul(msq[:], mean[:], mean[:])
                nc.vector.tensor_sub(var[:], var[:], msq[:])
                nc.scalar.activation(var[:], var[:], A.Sqrt, bias=eps_t[:])
                rs = temps.tile([P, SH], f32, tag="ln_rs")
                nc.vector.reciprocal(rs[:], var[:])
                for k in range(KT):
                    t = temps.tile([P, SH], f32, tag="ln_t")
                    nc.vector.tensor_sub(t[:], x_sb[:, k, :], mean[:])
                    nc.vector.tensor_mul(t[:], t[:], rs[:])
                    out_writer(k, t, w_t[:, k : k + 1], b_t[:, k : k + 1])

            def act_scale_shift(dst, src_ap, w, b):
                nc.scalar.activation(dst, src_ap, A.Identity, bias=b, scale=w)

            def ln_cmajor(x_sb, w_t, b_t, out_writer):
                s1 = psum_y.tile([P, SH], f32, tag="yaug")
                s2 = psum_y.tile([P, SH], f32, tag="yaug")
                for k in range(KT):
                    ln_stats_feed(s1, s2, x_sb[:, k, :], k)
                ln_finalize(s1, s2, x_sb, w_t, b_t, out_writer)

            wp_sb = persist.tile([P, KT, C], bf16)
            yfull = persist.tile([P, KT, SH], bf16)

            # ---------------- Phase 1: LN1 + AllGather + QKV + attention ----
            with (
                tc.tile_pool(name="ph1", bufs=1) as ph1,
                tc.tile_pool(name="hstream", bufs=4) as hstream,
                tc.tile_pool(name="ppool", bufs=8) as ppool,
                tc.tile_pool(name="psum_s", bufs=3, space="PSUM") as psum_s,
            ):
                def ln1_writer(k, t, w, b):
                    hk = temps.tile([P, SH], bf16, tag="ln_xbf")
                    act_scale_shift(hk[:], t[:], w, b)
                    nc.sync.dma_start(blocked(ag_in[:])[:, k, :], hk[:])

                ln_cmajor(xT_sb, lnp["ln1w"], lnp["ln1b"], ln1_writer)

                if stub_collectives:
                    # timing-sim stand-in for the AllGather (data is wrong,
                    # only the dependency structure matters; real AG ~15us
                    # runs on separate collective hardware)
                    for s in range(NC):
                        nc.sync.dma_start(ag_out[s * C : s * C + 2, :], ag_in[0:2, :])
                else:
                    nc.gpsimd.collective_compute(
                        "AllGather", mybir.AluOpType.bypass, replica_groups=rg,
                        ins=[ag_in.opt()], outs=[ag_out.opt()],
                    )

                wq_sb = ph1.tile([P, KT, P], bf16)
                nc.sync.dma_start(wq_sb[:], wqT[:])
                wk_sb = ph1.tile([P, KT, P], bf16)
                nc.sync.dma_start(wk_sb[:], wkT[:])
                wv_sb = ph1.tile([P, KT, P], bf16)
                nc.sync.dma_start(wv_sb[:], wvT[:])

                qT_sb = ph1.tile([P, NC, SH], bf16)
                kT_sb = ph1.tile([P, NC, SH], bf16)
                # v token-major, augmented with 64 ones-columns per head
                v_aug = ph1.tile([P, TTILES, 4, HD], bf16)
                nc.vector.memset(v_aug[:, :, 1, :], 1.0)
                nc.vector.memset(v_aug[:, :, 3, :], 1.0)
                for g in range(NC):
                    # one pass over h for q, k and v of this 512-token chunk
                    h_g = hstream.tile([P, KT, SH], bf16, tag="hg")
                    heng = nc.scalar if g < 4 else nc.sync
                    heng.dma_start(h_g[:], blocked(ag_out[g * C : (g + 1) * C, :]))
                    pqk = psum_s.tile([P, 2, SH], f32, tag="spair")
                    for k in range(KT):
                        nc.tensor.matmul(pqk[:, 0, :], wq_sb[:, k, :], h_g[:, k, :], start=(k == 0), stop=(k == KT - 1))
                        nc.tensor.matmul(pqk[:, 1, :], wk_sb[:, k, :], h_g[:, k, :], start=(k == 0), stop=(k == KT - 1))
                    nc.vector.tensor_scalar(qT_sb[:, g, :], pqk[:, 0, :], bqk_t[:, 0:1], None, OP.add)
                    nc.vector.tensor_scalar(kT_sb[:, g, :], pqk[:, 1, :], bqk_t[:, 1:2], None, OP.add)
                    pv2 = psum_s.tile([P, 2, SH], f32, tag="spair")
                    for jj in range(4):
                        j = 4 * g + jj
                        psv = pv2[:, jj // 2, (jj % 2) * P : (jj % 2) * P + P]
                        for k in range(KT):
                            nc.tensor.matmul(
                                psv,
                                h_g[:, k, jj * P : (jj + 1) * P],
                                wv_sb[:, k, :],
                                start=(k == 0), stop=(k == KT - 1),
                            )
                        nc.vector.tensor_tensor(
                            v_aug[:, j, 0::2, :],
                            psv.rearrange("p (hh x) -> p hh x", x=HD),
                            bv_rep.rearrange("p (hh x) -> p hh x", x=HD),
                            OP.add,
                        )

                nc.scalar.dma_start(wp_sb[:], wpT[:])

                # attention per (batch, q-chunk); 2 heads per core
                yT_sb = ph1.tile([P, NC, SH], bf16)
                # heaviest q-chunks first: the attention tail before the
                # AllToAll is then the lightest chunk's chain
                for g in (3, 7, 2, 6, 1, 5, 0, 4):
                    b, qc = g // CPB, g % CPB
                    n_kt = 4 * (qc + 1)
                    ya0 = psum_y.tile([P, SH], f32, tag="yaug")
                    ya1 = psum_y.tile([P, SH], f32, tag="yaug")
                    for kp in range(n_kt // 2):
                        kt0, kt1 = 2 * kp, 2 * kp + 1
                        s0 = psum_s.tile([P, 2, SH], f32, tag="spair")
                        s1 = psum_s.tile([P, 2, SH], f32, tag="spair")
                        for i, kt in enumerate((kt0, kt1)):
                            ksl = (b * CPB + kt // 4, slice((kt % 4) * P, (kt % 4 + 1) * P))
                            nc.tensor.matmul(s0[:, i, :], kT_sb[0:HD, ksl[0], ksl[1]], qT_sb[0:HD, g, :], start=True, stop=True)
                            nc.tensor.matmul(s1[:, i, :], kT_sb[HD:P, ksl[0], ksl[1]], qT_sb[HD:P, g, :], start=True, stop=True)
                        p0 = ppool.tile([P, 2, SH], bf16, tag="pt")
                        p1 = ppool.tile([P, 2, SH], bf16, tag="pt")
                        nc.scalar.activation(p0[:], s0[:], A.Exp, scale=1.0 / np.sqrt(HD))
                        nc.scalar.activation(p1[:], s1[:], A.Exp, scale=1.0 / np.sqrt(HD))
                        for i, kt in enumerate((kt0, kt1)):
                            d = kt - 4 * qc
                            if d >= 0:
                                nc.vector.tensor_mul(p0[:, i, :], p0[:, i, :], mask_t[:, d, :])
                                nc.vector.tensor_mul(p1[:, i, :], p1[:, i, :], mask_t[:, d, :])
                        for i, kt in enumerate((kt0, kt1)):
                            j = 16 * b + kt
                            nc.tensor.matmul(ya0[:], v_aug[:, j, 0:2, :].rearrange("p a b -> p (a b)"), p0[:, i, :], start=(kt == 0), stop=(kt == n_kt - 1))
                            nc.tensor.matmul(ya1[:], v_aug[:, j, 2:4, :].rearrange("p a b -> p (a b)"), p1[:, i, :], start=(kt == 0), stop=(kt == n_kt - 1))
                    rec0 = temps.tile([P, SH], f32, tag="rec")
                    nc.vector.reciprocal(rec0[HD:P, :], ya0[HD:P, :])
                    nc.vector.tensor_tensor(yT_sb[0:HD, g, :], ya0[0:HD, :], rec0[HD:P, :], OP.mult)
                    rec1 = temps.tile([P, SH], f32, tag="rec")
                    nc.vector.reciprocal(rec1[HD:P, :], ya1[HD:P, :])
                    nc.vector.tensor_tensor(yT_sb[HD:P, g, :], ya1[0:HD, :], rec1[HD:P, :], OP.mult)
                    nc.sync.dma_start(a2a_in[g * P : (g + 1) * P, :], yT_sb[:, g, :])

            # ---------------- Phase 2: A2A + Wp + LN2 + MLP -----------------
            with (
                tc.tile_pool(name="ph3", bufs=1) as ph3,
                tc.tile_pool(name="w1p", bufs=6) as w1p,
                tc.tile_pool(name="w2p", bufs=3) as w2p,
                tc.tile_pool(name="psum_t", bufs=4, space="PSUM") as psum_t,
            ):
                if stub_collectives:
                    nc.sync.dma_start(a2a_out[0:2, :], a2a_in[0:2, :])
                else:
                    nc.gpsimd.collective_compute(
                        "AllToAll", mybir.AluOpType.bypass, replica_groups=rg,
                        ins=[a2a_in.opt()], outs=[a2a_out.opt()],
                    )
                for k in range(KT):
                    nc.sync.dma_start(yfull[:, k, :], a2a_out[k * P : (k + 1) * P, :])

                x2T = ph3.tile([P, KT, SH], f32)
                ls1 = psum_t.tile([P, SH], f32, tag="pst")
                ls2 = psum_t.tile([P, SH], f32, tag="pst")
                for m in range(KT):
                    ps = psum_t.tile([P, SH], f32, tag="pst")
                    for k in range(KT):
                        nc.tensor.matmul(ps[:], wp_sb[:, k, m * P : (m + 1) * P], yfull[:, k, :], start=(k == 0), stop=(k == KT - 1))
                    t = temps.tile([P, SH], f32, tag="ev")
                    nc.scalar.activation(t[:], ps[:], A.Identity, bias=bp_t[:, m : m + 1])
                    nc.vector.tensor_add(x2T[:, m, :], t[:], xT_sb[:, m, :])
                    ln_stats_feed(ls1, ls2, x2T[:, m, :], m)

                h2T = ph3.tile([P, KT, SH], bf16)

                def ln2_writer(k, t, w, b):
                    act_scale_shift(h2T[:, k, :], t[:], w, b)

                ln_finalize(ls1, ls2, x2T, lnp["ln2w"], lnp["ln2b"], ln2_writer)

                def w2_evict(m, ps):
                    of = temps.tile([P, SH], f32, tag="ev")
                    nc.scalar.activation(of[:], ps[:], A.Identity, bias=b2_t[:, m : m + 1])
                    nc.vector.tensor_add(of[:], of[:], x2T[:, m, :])
                    nc.sync.dma_start(blocked(outT)[:, m, :], of[:])

                mT = ph3.tile([P, FT, SH], bf16)
                for fidx in range(FT):
                    w1t = w1p.tile([P, KT, P], bf16, tag="w1t")
                    nc.sync.dma_start(w1t[:], w1T[fidx])
                    ps = psum_t.tile([P, SH], f32, tag="pst")
                    for k in range(KT):
                        nc.tensor.matmul(ps[:], w1t[:, k, :], h2T[:, k, :], start=(k == 0), stop=(k == KT - 1))
                    nc.scalar.activation(mT[:, fidx, :], ps[:], A.Gelu, bias=b1_t[:, fidx : fidx + 1])

                for m in range(KT):
                    w2t = w2p.tile([P, FT, P], bf16, tag="w2t")
                    nc.sync.dma_start(w2t[:], w2T[m])
                    ps = psum_t.tile([P, SH], f32, tag="pst")
                    for k in range(FT):
                        nc.tensor.matmul(ps[:], w2t[:, k, :], mT[:, k, :], start=(k == 0), stop=(k == FT - 1))
                    w2_evict(m, ps)

    nc.compile()
    return nc


def _prep_inputs(inputs):
    x = np.asarray(inputs["x"], np.float32)
    x2d = np.ascontiguousarray(x.reshape(B * T, C))
    xT_full = np.ascontiguousarray(x2d.T)  # [C, B*T]

    Wq = np.asarray(inputs["Wq"], np.float32)
    Wk = np.asarray(inputs["Wk"], np.float32)
    Wv = np.asarray(inputs["Wv"], np.float32)
    Wp = np.asarray(inputs["Wp"], np.float32)
    W1 = np.asarray(inputs["W1"], np.float32)
    W2 = np.asarray(inputs["W2"], np.float32)

    def block_k(a, dt):
        # [KO*P, M] -> [P, KO, M]   (row r = 128*ko + ki)
        ko = a.shape[0] // P
        return np.ascontiguousarray(a.reshape(ko, P, a.shape[1]).transpose(1, 0, 2)).astype(dt)

    wpT = block_k(Wp.T, BF)                               # [P, KT, C]
    w1T_f = W1.T                                          # [C, FF]
    w1T = np.ascontiguousarray(
        np.stack([block_k(w1T_f[:, f * P : (f + 1) * P], np.float32) for f in range(FT)])
    ).astype(BF)                                          # [FT, P, KT, P]
    w2T_f = W2.T                                          # [FF, C]
    w2T = np.ascontiguousarray(
        np.stack([block_k(w2T_f[:, m * P : (m + 1) * P], np.float32) for m in range(KT)])
    ).astype(BF)                                          # [KT, P, FT, P]

    def pack_pcol(v, nt):  # [nt*P] -> [P, nt]
        return np.ascontiguousarray(np.asarray(v, np.float32).reshape(nt, P).T)

    bp = pack_pcol(inputs["bp"], KT)
    b1 = pack_pcol(inputs["b1"], FT)
    b2 = pack_pcol(inputs["b2"], KT)
    ln1w = pack_pcol(inputs["ln1_w"], KT)
    ln1b = pack_pcol(inputs["ln1_b"], KT)
    ln2w = pack_pcol(inputs["ln2_w"], KT)
    ln2b = pack_pcol(inputs["ln2_b"], KT)

    i_idx = np.arange(P)[:, None, None]
    d_idx = np.arange(CPB)[None, :, None]
    j_idx = np.arange(SH)[None, None, :]
    masks = ((P * d_idx + i_idx) <= j_idx).astype(BF)

    bq = np.asarray(inputs["bq"], np.float32)
    bk = np.asarray(inputs["bk"], np.float32)
    bvv = np.asarray(inputs["bv"], np.float32)

    in_maps = []
    for c in range(NC):
        rs = slice(P * c, P * (c + 1))
        m = {
            "xT": block_k(xT_full[:, SH * c : SH * (c + 1)], np.float32),
            "wqT": block_k(Wq[rs, :].T, BF),
            "wkT": block_k(Wk[rs, :].T, BF),
            "wvT": block_k(Wv[rs, :].T, BF),
            "wpT": wpT,
            "w1T": w1T,
            "w2T": w2T,
            "bqk": np.ascontiguousarray(np.stack([bq[rs], bk[rs]], axis=1)),
            "bv": np.ascontiguousarray(bvv[rs][None, :]),
            "bp": bp, "b1": b1, "b2": b2,
            "ln1w": ln1w, "ln1b": ln1b, "ln2w": ln2w, "ln2b": ln2b,
            "masks": masks,
        }
        in_maps.append(m)
    return in_maps


def kernel(**inputs):
    if "nc" not in _CACHE:
        _CACHE["nc"] = _build()
    nc = _CACHE["nc"]
    in_maps = _prep_inputs(inputs)
    res = bass_utils.run_bass_kernel_spmd(nc, in_maps, core_ids=list(range(NC)))
    out2d = np.empty((B * T, C), np.float32)
    for c in range(NC):
        out2d[SH * c : SH * (c + 1), :] = res.results[c]["outT"].T
    return out2d.reshape(B, T, C)


# revision 45
# speedup vs baseline: 1.2245x; 1.2245x over previous
"""Trainium2 Bass kernel for a dense transformer block (B=2, T=2048, C=1024, H=16).

Sharding across 8 NeuronCores:
  - LayerNorm1 computed token-sharded (512 tokens/core), AllGather of h.
  - Attention tensor-parallel over heads (2 heads/core): QKV projections,
    causal softmax, P@V all local per head.
  - AllToAll converts head-sharded y to token-sharded full-channel y.
  - Output projection Wp, LayerNorm2 and the whole MLP are token-sharded
    (full weights per core), so no further communication is needed.

Everything on-chip is kept channel-major ([C, tokens], C on partitions) so no
transposes are ever needed: weights are pre-transposed on the host, LayerNorm
statistics are computed with an all-ones matmul on the tensor engine (which
also broadcasts the per-token stats across all 128 partitions for free), and
the softmax denominator comes from 64 replicated ones-columns appended to V.
Matmul inputs are bf16 (fp32 accumulation in PSUM); the residual path stays
fp32 end to end.
"""

import sys

sys.path.insert(0, "/opt/trn_rl_repo")

import numpy as np
import ml_dtypes

import concourse.bass as bass
import concourse.bacc as bacc
import concourse.tile as tile
import concourse.mybir as mybir
from concourse import bass_utils

B, T, C, H = 2, 2048, 1024, 16
HD = C // H          # 64
FF = 4 * C           # 4096
EPS = 1e-5
NC = 8               # cores
P = 128
SH = (B * T) // NC   # 512 tokens per shard
KT = C // P          # 8 k-subtiles over C
FT = FF // P         # 32 ff tiles
TTILES = (B * T) // P  # 32 global 128-token tiles
CPB = T // SH        # 4 chunks per batch
f32 = mybir.dt.float32
bf16 = mybir.dt.bfloat16
BF = ml_dtypes.bfloat16

_CACHE = {}


def _build(stub_collectives=False):
    nc = bacc.Bacc("TRN2", target_bir_lowering=False, debug=False,
                   num_devices=1 if stub_collectives else NC)
    A = mybir.ActivationFunctionType
    OP = mybir.AluOpType

    def dram_in(name, shape, dt):
        return nc.dram_tensor(name, shape, dt, kind="ExternalInput").ap()

    xT = dram_in("xT", [P, KT, SH], f32)      # C-major token shard, k-blocked
    wqT = dram_in("wqT", [P, KT, P], bf16)    # [ki, ko, M=128 q-ch] blocked
    wkT = dram_in("wkT", [P, KT, P], bf16)
    wvT = dram_in("wvT", [P, KT, P], bf16)
    wpT = dram_in("wpT", [P, KT, C], bf16)    # [ki, ko, M=C] blocked (full)
    w1T = dram_in("w1T", [FT, P, KT, P], bf16)  # per ff-tile: [ki, ko, 128]
    w2T = dram_in("w2T", [KT, P, FT, P], bf16)  # per C-tile: [ki, ko, 128]
    bqk = dram_in("bqk", [P, 2], f32)         # [:,0]=bq slice, [:,1]=bk slice
    bv = dram_in("bv", [1, P], f32)           # bv slice (free-axis add)
    bp = dram_in("bp", [P, KT], f32)
    b1 = dram_in("b1", [P, FT], f32)
    b2 = dram_in("b2", [P, KT], f32)
    ln1w = dram_in("ln1w", [P, KT], f32)
    ln1b = dram_in("ln1b", [P, KT], f32)
    ln2w = dram_in("ln2w", [P, KT], f32)
    ln2b = dram_in("ln2b", [P, KT], f32)
    masks = dram_in("masks", [P, CPB, SH], bf16)  # [i, d, j] = (128d+i) <= j

    outT = nc.dram_tensor("outT", [C, SH], f32, kind="ExternalOutput").ap()

    rg = [list(range(NC))]

    def blocked(ap, ki=P):
        # [ (ko ki), s ] -> [ ki, ko, s ]  (channel c = 128*ko + ki)
        return ap.rearrange("(ko ki) s -> ki ko s", ki=ki)

    with tile.TileContext(nc) as tc:
        with (
            tc.tile_pool(name="dram", bufs=1, space="DRAM") as dram,
            tc.tile_pool(name="const", bufs=1) as const,
            tc.tile_pool(name="persist", bufs=1) as persist,
            tc.tile_pool(name="temps", bufs=3) as temps,
            tc.tile_pool(name="psum_y", bufs=2, space="PSUM") as psum_y,
        ):
            ag_in = dram.tile([C, SH], bf16)
            ag_out = dram.tile([NC * C, SH], bf16,
                               addr_space="Local" if stub_collectives else "Shared")
            a2a_in = dram.tile([NC * P, SH], bf16)
            a2a_out = dram.tile([NC * P, SH], bf16)

            ones_bf = const.tile([P, P], bf16)
            nc.vector.memset(ones_bf[:], 1.0)
            eps_t = const.tile([P, 1], f32)
            nc.vector.memset(eps_t[:], EPS)
            bqk_t = const.tile([P, 2], f32)
            nc.sync.dma_start(bqk_t[:], bqk[:])
            bv_rep = const.tile([P, P], f32)
            nc.sync.dma_start(
                bv_rep[:],
                bass.AP(tensor=bv.tensor, offset=bv.offset, ap=[[0, P], [1, P]]),
            )
            bp_t = const.tile([P, KT], f32)
            nc.sync.dma_start(bp_t[:], bp[:])
            b1_t = const.tile([P, FT], f32)
            nc.sync.dma_start(b1_t[:], b1[:])
            b2_t = const.tile([P, KT], f32)
            nc.sync.dma_start(b2_t[:], b2[:])
            lnp = {}
            for nm, ap in (("ln1w", ln1w), ("ln1b", ln1b), ("ln2w", ln2w), ("ln2b", ln2b)):
                t = const.tile([P, KT], f32, tag=nm)
                nc.sync.dma_start(t[:], ap[:])
                lnp[nm] = t
            mask_t = const.tile([P, CPB, SH], bf16)
            nc.scalar.dma_start(mask_t[:], masks[:])

            xT_sb = persist.tile([P, KT, SH], f32)
            for k in range(KT):
                nc.sync.dma_start(xT_sb[:, k, :], xT[:, k, :])

            def ln_stats_feed(s1, s2, x_ap, k):
                """Feed one [P, SH] fp32 tile into the LN stat accumulators.

                All-ones matmuls both sum over the C partition axis and
                broadcast the per-token result to every partition of the
                PSUM accumulators."""
                xbf = temps.tile([P, SH], bf16, tag="ln_xbf")
                nc.scalar.activation(xbf[:], x_ap, A.Copy)
                nc.tensor.matmul(s1[:], ones_bf[:], xbf[:], start=(k == 0), stop=(k == KT - 1))
                sq = temps.tile([P, SH], bf16, tag="ln_sq")
                nc.vector.tensor_mul(sq[:], xbf[:], xbf[:])
                nc.tensor.matmul(s2[:], ones_bf[:], sq[:], start=(k == 0), stop=(k == KT - 1))

            def ln_finalize(s1, s2, x_sb, w_t, b_t, out_writer):
                mean = temps.tile([P, SH], f32, tag="ln_mean")
                nc.vector.tensor_scalar_mul(mean[:], s1[:], 1.0 / C)
                var = temps.tile([P, SH], f32, tag="ln_var")
                nc.vector.tensor_scalar_mul(var[:], s2[:], 1.0 / C)
                msq = temps.tile([P, SH], f32, tag="ln_t")
                nc.vector.tensor_mul(msq[:], mean[:], mean[:])
                nc.vector.tensor_sub(var[:], var[:], msq[:])
                nc.scalar.activation(var[:], var[:], A.Sqrt, bias=eps_t[:])
                rs = temps.tile([P, SH], f32, tag="ln_rs")
                nc.vector.reciprocal(rs[:], var[:])
                for k in range(KT):
                    t = temps.tile([P, SH], f32, tag="ln_t")
                    nc.vector.tensor_sub(t[:], x_sb[:, k, :], mean[:])
                    nc.vector.tensor_mul(t[:], t[:], rs[:])
                    out_writer(k, t, w_t[:, k : k + 1], b_t[:, k : k + 1])

            def act_scale_shift(dst, src_ap, w, b):
                nc.scalar.activation(dst, src_ap, A.Identity, bias=b, scale=w)

            def ln_cmajor(x_sb, w_t, b_t, out_writer):
                s1 = psum_y.tile([P, SH], f32, tag="yaug")
                s2 = psum_y.tile([P, SH], f32, tag="yaug")
                for k in range(KT):
                    ln_stats_feed(s1, s2, x_sb[:, k, :], k)
                ln_finalize(s1, s2, x_sb, w_t, b_t, out_writer)

            wp_sb = persist.tile([P, KT, C], bf16)
            yfull = persist.tile([P, KT, SH], bf16)

            # ---------------- Phase 1: LN1 + AllGather + QKV + attention ----
            with (
                tc.tile_pool(name="ph1", bufs=1) as ph1,
                tc.tile_pool(name="hstream", bufs=4) as hstream,
                tc.tile_pool(name="ppool", bufs=8) as ppool,
                tc.tile_pool(name="psum_s", bufs=3, space="PSUM") as psum_s,
            ):
                def ln1_writer(k, t, w, b):
                    hk = temps.tile([P, SH], bf16, tag="ln_xbf")
                    act_scale_shift(hk[:], t[:], w, b)
                    nc.sync.dma_start(blocked(ag_in[:])[:, k, :], hk[:])

                ln_cmajor(xT_sb, lnp["ln1w"], lnp["ln1b"], ln1_writer)

                if stub_collectives:
                    # timing-sim stand-in for the AllGather (data is wrong,
                    # only the dependency structure matters; real AG ~15us
                    # runs on separate collective hardware)
                    for s in range(NC):
                        nc.sync.dma_start(ag_out[s * C : s * C + 2, :], ag_in[0:2, :])
                else:
                    nc.gpsimd.collective_compute(
                        "AllGather", mybir.AluOpType.bypass, replica_groups=rg,
                        ins=[ag_in.opt()], outs=[ag_out.opt()],
                    )

                wq_sb = ph1.tile([P, KT, P], bf16)
                nc.sync.dma_start(wq_sb[:], wqT[:])
                wk_sb = ph1.tile([P, KT, P], bf16)
                nc.sync.dma_start(wk_sb[:], wkT[:])
                wv_sb = ph1.tile([P, KT, P], bf16)
                nc.sync.dma_start(wv_sb[:], wvT[:])

                qT_sb = ph1.tile([P, NC, SH], bf16)
                kT_sb = ph1.tile([P, NC, SH], bf16)
                # v token-major, augmented with 64 ones-columns per head
                v_aug = ph1.tile([P, TTILES, 4, HD], bf16)
                nc.vector.memset(v_aug[:, :, 1, :], 1.0)
                nc.vector.memset(v_aug[:, :, 3, :], 1.0)
                for g in range(NC):
                    # one pass over h for q, k and v of this 512-token chunk
                    h_g = hstream.tile([P, KT, SH], bf16, tag="hg")
                    heng = nc.scalar if g < 4 else nc.sync
                    heng.dma_start(h_g[:], blocked(ag_out[g * C : (g + 1) * C, :]))
                    pqk = psum_s.tile([P, 2, SH], f32, tag="spair")
                    for k in range(KT):
                        nc.tensor.matmul(pqk[:, 0, :], wq_sb[:, k, :], h_g[:, k, :], start=(k == 0), stop=(k == KT - 1))
                        nc.tensor.matmul(pqk[:, 1, :], wk_sb[:, k, :], h_g[:, k, :], start=(k == 0), stop=(k == KT - 1))
                    nc.vector.tensor_scalar(qT_sb[:, g, :], pqk[:, 0, :], bqk_t[:, 0:1], None, OP.add)
                    nc.vector.tensor_scalar(kT_sb[:, g, :], pqk[:, 1, :], bqk_t[:, 1:2], None, OP.add)
                    pv2 = psum_s.tile([P, 2, SH], f32, tag="spair")
                    for jj in range(4):
                        j = 4 * g + jj
                        psv = pv2[:, jj // 2, (jj % 2) * P : (jj % 2) * P + P]
                        for k in range(KT):
                            nc.tensor.matmul(
                                psv,
                                h_g[:, k, jj * P : (jj + 1) * P],
                                wv_sb[:, k, :],
                                start=(k == 0), stop=(k == KT - 1),
                            )
                        nc.vector.tensor_tensor(
                            v_aug[:, j, 0::2, :],
                            psv.rearrange("p (hh x) -> p hh x", x=HD),
                            bv_rep.rearrange("p (hh x) -> p hh x", x=HD),
                            OP.add,
                        )

                nc.scalar.dma_start(wp_sb[:], wpT[:])

                # attention per (batch, q-chunk); 2 heads per core
                yT_sb = ph1.tile([P, NC, SH], bf16)
                # heaviest q-chunks first: the attention tail before the
                # AllToAll is then the lightest chunk's chain
                for g in (3, 7, 2, 6, 1, 5, 0, 4):
                    b, qc = g // CPB, g % CPB
                    n_kt = 4 * (qc + 1)
                    ya0 = psum_y.tile([P, SH], f32, tag="yaug")
                    ya1 = psum_y.tile([P, SH], f32, tag="yaug")
                    for kp in range(n_kt // 2):
                        kt0, kt1 = 2 * kp, 2 * kp + 1
                        s0 = psum_s.tile([P, 2, SH], f32, tag="spair")
                        s1 = psum_s.tile([P, 2, SH], f32, tag="spair")
                        for i, kt in enumerate((kt0, kt1)):
                            ksl = (b * CPB + kt // 4, slice((kt % 4) * P, (kt % 4 + 1) * P))
                            nc.tensor.matmul(s0[:, i, :], kT_sb[0:HD, ksl[0], ksl[1]], qT_sb[0:HD, g, :], start=True, stop=True)
                            nc.tensor.matmul(s1[:, i, :], kT_sb[HD:P, ksl[0], ksl[1]], qT_sb[HD:P, g, :], start=True, stop=True)
                        p0 = ppool.tile([P, 2, SH], bf16, tag="pt")
                        p1 = ppool.tile([P, 2, SH], bf16, tag="pt")
                        nc.scalar.activation(p0[:], s0[:], A.Exp, scale=1.0 / np.sqrt(HD))
                        nc.scalar.activation(p1[:], s1[:], A.Exp, scale=1.0 / np.sqrt(HD))
                        for i, kt in enumerate((kt0, kt1)):
                            d = kt - 4 * qc
                            if d >= 0:
                                nc.vector.tensor_mul(p0[:, i, :], p0[:, i, :], mask_t[:, d, :])
                                nc.vector.tensor_mul(p1[:, i, :], p1[:, i, :], mask_t[:, d, :])
                        for i, kt in enumerate((kt0, kt1)):
                            j = 16 * b + kt
                            nc.tensor.matmul(ya0[:], v_aug[:, j, 0:2, :].rearrange("p a b -> p (a b)"), p0[:, i, :], start=(kt == 0), stop=(kt == n_kt - 1))
                            nc.tensor.matmul(ya1[:], v_aug[:, j, 2:4, :].rearrange("p a b -> p (a b)"), p1[:, i, :], start=(kt == 0), stop=(kt == n_kt - 1))
                    rec0 = temps.tile([P, SH], f32, tag="rec")
                    nc.vector.reciprocal(rec0[HD:P, :], ya0[HD:P, :])
                    nc.vector.tensor_tensor(yT_sb[0:HD, g, :], ya0[0:HD, :], rec0[HD:P, :], OP.mult)
                    rec1 = temps.tile([P, SH], f32, tag="rec")
                    nc.vector.reciprocal(rec1[HD:P, :], ya1[HD:P, :])
                    nc.vector.tensor_tensor(yT_sb[HD:P, g, :], ya1[0:HD, :], rec1[HD:P, :], OP.mult)
                    nc.sync.dma_start(a2a_in[g * P : (g + 1) * P, :], yT_sb[:, g, :])

            # ---------------- Phase 2: A2A + Wp + LN2 + MLP -----------------
            with (
                tc.tile_pool(name="ph3", bufs=1) as ph3,
                tc.tile_pool(name="w1p", bufs=6) as w1p,
                tc.tile_pool(name="w2p", bufs=3) as w2p,
                tc.tile_pool(name="psum_t", bufs=4, space="PSUM") as psum_t,
            ):
                if stub_collectives:
                    nc.sync.dma_start(a2a_out[0:2, :], a2a_in[0:2, :])
                else:
                    nc.gpsimd.collective_compute(
                        "AllToAll", mybir.AluOpType.bypass, replica_groups=rg,
                        ins=[a2a_in.opt()], outs=[a2a_out.opt()],
                    )
                for k in range(KT):
                    nc.sync.dma_start(yfull[:, k, :], a2a_out[k * P : (k + 1) * P, :])

                x2T = ph3.tile([P, KT, SH], f32)
                ls1 = psum_t.tile([P, SH], f32, tag="pst")
                ls2 = psum_t.tile([P, SH], f32, tag="pst")
                for m in range(KT):
                    ps = psum_t.tile([P, SH], f32, tag="pst")
                    for k in range(KT):
                        nc.tensor.matmul(ps[:], wp_sb[:, k, m * P : (m + 1) * P], yfull[:, k, :], start=(k == 0), stop=(k == KT - 1))
                    t = temps.tile([P, SH], f32, tag="ev")
                    nc.scalar.activation(t[:], ps[:], A.Identity, bias=bp_t[:, m : m + 1])
                    nc.vector.tensor_add(x2T[:, m, :], t[:], xT_sb[:, m, :])
                    ln_stats_feed(ls1, ls2, x2T[:, m, :], m)

                h2T = ph3.tile([P, KT, SH], bf16)

                def ln2_writer(k, t, w, b):
                    act_scale_shift(h2T[:, k, :], t[:], w, b)

                ln_finalize(ls1, ls2, x2T, lnp["ln2w"], lnp["ln2b"], ln2_writer)

                def w2_evict(m, ps):
                    of = temps.tile([P, SH], f32, tag="ev")
                    nc.scalar.activation(of[:], ps[:], A.Identity, bias=b2_t[:, m : m + 1])
                    nc.vector.tensor_add(of[:], of[:], x2T[:, m, :])
                    nc.sync.dma_start(blocked(outT)[:, m, :], of[:])

                mT = ph3.tile([P, FT, SH], bf16)
                for fidx in range(FT):
                    w1t = w1p.tile([P, KT, P], bf16, tag="w1t")
                    nc.sync.dma_start(w1t[:], w1T[fidx])
                    ps = psum_t.tile([P, SH], f32, tag="pst")
                    for k in range(KT):
                        nc.tensor.matmul(ps[:], w1t[:, k, :], h2T[:, k, :], start=(k == 0), stop=(k == KT - 1))
                    nc.scalar.activation(mT[:, fidx, :], ps[:], A.Gelu, bias=b1_t[:, fidx : fidx + 1])

                for m in range(KT):
                    w2t = w2p.tile([P, FT, P], bf16, tag="w2t")
                    nc.sync.dma_start(w2t[:], w2T[m])
                    ps = psum_t.tile([P, SH], f32, tag="pst")
                    for k in range(FT):
                        nc.tensor.matmul(ps[:], w2t[:, k, :], mT[:, k, :], start=(k == 0), stop=(k == FT - 1))
                    w2_evict(m, ps)

    nc.compile()
    return nc


def _prep_inputs(inputs):
    x = np.asarray(inputs["x"], np.float32)
    x2d = np.ascontiguousarray(x.reshape(B * T, C))
    xT_full = np.ascontiguousarray(x2d.T)  # [C, B*T]

    Wq = np.asarray(inputs["Wq"], np.float32)
    Wk = np.asarray(inputs["Wk"], np.float32)
    Wv = np.asarray(inputs["Wv"], np.float32)
    Wp = np.asarray(inputs["Wp"], np.float32)
    W1 = np.asarray(inputs["W1"], np.float32)
    W2 = np.asarray(inputs["W2"], np.float32)

    def block_k(a, dt):
        # [KO*P, M] -> [P, KO, M]   (row r = 128*ko + ki)
        ko = a.shape[0] // P
        return np.ascontiguousarray(a.reshape(ko, P, a.shape[1]).transpose(1, 0, 2)).astype(dt)

    wpT = block_k(Wp.T, BF)                               # [P, KT, C]
    w1T_f = W1.T                                          # [C, FF]
    w1T = np.ascontiguousarray(
        np.stack([block_k(w1T_f[:, f * P : (f + 1) * P], np.float32) for f in range(FT)])
    ).astype(BF)                                          # [FT, P, KT, P]
    w2T_f = W2.T                                          # [FF, C]
    w2T = np.ascontiguousarray(
        np.stack([block_k(w2T_f[:, m * P : (m + 1) * P], np.float32) for m in range(KT)])
    ).astype(BF)                                          # [KT, P, FT, P]

    def pack_pcol(v, nt):  # [nt*P] -> [P, nt]
        return np.ascontiguousarray(np.asarray(v, np.float32).reshape(nt, P).T)

    bp = pack_pcol(inputs["bp"], KT)
    b1 = pack_pcol(inputs["b1"], FT)
    b2 = pack_pcol(inputs["b2"], KT)
    ln1w = pack_pcol(inputs["ln1_w"], KT)
    ln1b = pack_pcol(inputs["ln1_b"], KT)
    ln2w = pack_pcol(inputs["ln2_w"], KT)
    ln2b = pack_pcol(inputs["ln2_b"], KT)

    i_idx = np.arange(P)[:, None, None]
    d_idx = np.arange(CPB)[None, :, None]
    j_idx = np.arange(SH)[None, None, :]
    masks = ((P * d_idx + i_idx) <= j_idx).astype(BF)

    bq = np.asarray(inputs["bq"], np.float32)
    bk = np.asarray(inputs["bk"], np.float32)
    bvv = np.asarray(inputs["bv"], np.float32)

    in_maps = []
    for c in range(NC):
        rs = slice(P * c, P * (c + 1))
        m = {
            "xT": block_k(xT_full[:, SH * c : SH * (c + 1)], np.float32),
            "wqT": block_k(Wq[rs, :].T, BF),
            "wkT": block_k(Wk[rs, :].T, BF),
            "wvT": block_k(Wv[rs, :].T, BF),
            "wpT": wpT,
            "w1T": w1T,
            "w2T": w2T,
            "bqk": np.ascontiguousarray(np.stack([bq[rs], bk[rs]], axis=1)),
            "bv": np.ascontiguousarray(bvv[rs][None, :]),
            "bp": bp, "b1": b1, "b2": b2,
            "ln1w": ln1w, "ln1b": ln1b, "ln2w": ln2w, "ln2b": ln2b,
            "masks": masks,
        }
        in_maps.append(m)
    return in_maps


def kernel(**inputs):
    if "nc" not in _CACHE:
        _CACHE["nc"] = _build()
    nc = _CACHE["nc"]
    in_maps = _prep_inputs(inputs)
    res = bass_utils.run_bass_kernel_spmd(nc, in_maps, core_ids=list(range(NC)))
    out2d = np.empty((B * T, C), np.float32)
    for c in range(NC):
        out2d[SH * c : SH * (c + 1), :] = res.results[c]["outT"].T
    return out2d.reshape(B, T, C)
